# revision 1
# baseline (speedup 1.0000x reference)
"""Distributed causal multi-head attention for TRN2 (8 NeuronCores).

Problem: x[4,2048,1024] -> qkv proj (c_attn) -> 16-head causal attention
         -> output proj (c_proj).  N_HEAD=16, hd=64.

Sharding (zero collectives): core c handles batch b=c//2 and head-group
hg=c%2 (8 heads).  Each core computes q^T,k^T (transposed layout),
v (natural layout), causal attention with scores computed transposed
(S^T[j,i] = k_j . q_i), softmax without max-subtraction (inputs bounded;
masked future tiles skipped entirely, diagonal tiles masked by a post-exp
0/1 multiply), PV via an appended ones-column in V giving row sums for
free, then the c_proj partial product in transposed layout out^T[e,s].
Host sums the two head-group partials per batch, transposes, adds bias.
"""

import sys

if "/opt/trn_rl_repo" not in sys.path:
    sys.path.insert(0, "/opt/trn_rl_repo")

import numpy as np
import ml_dtypes

import concourse.bass as bass
import concourse.mybir as mybir
import concourse.tile as tile
from concourse import bacc
from concourse.bass_utils import run_bass_kernel_spmd

BF16 = mybir.dt.bfloat16
F32 = mybir.dt.float32

S = 2048            # sequence length
D = 1024            # model dim
H = 16              # total heads
HLOC = 8            # heads per core
HD = 64             # head dim
DQK = HLOC * HD     # 512: per-core q (or k) width
NDC = D // 128      # 8 d-chunks (contraction for qkv)
NSS = S // 512      # 4 i-supertiles
NST = S // 128      # 16 s-tiles / j-chunks
NRC = DQK // 128    # 4 contraction chunks for proj

_CACHED = {}


def _build(niter=1, phases="bcd"):
    nc = bacc.Bacc()

    xt_e = nc.declare_dram_parameter("xt", [D, S], BF16, isOutput=False)
    wqk_e = nc.declare_dram_parameter("wqk", [D, 2 * DQK], BF16, isOutput=False)
    wv_e = nc.declare_dram_parameter("wv", [D, DQK], BF16, isOutput=False)
    wp_e = nc.declare_dram_parameter("wp", [DQK, D], BF16, isOutput=False)
    bqk_e = nc.declare_dram_parameter("bqk", [128, 8], F32, isOutput=False)
    # wide diagonal masks: g=0 covers j-chunk offsets 0,1; g=1 covers 2,3
    msk_e = nc.declare_dram_parameter("msk", [2, 128, 1024], BF16, isOutput=False)
    out_e = nc.declare_dram_parameter("outT", [D, S], F32, isOutput=True)

    with tile.TileContext(nc) as tc:
        with tc.tile_pool(name="res", bufs=1) as res:
            # resident SBUF tensors
            xt = [res.tile([128, S], BF16, tag=f"xt{i}", name=f"xt{i}") for i in range(NDC)]
            wqk = [res.tile([128, 2 * DQK], BF16, tag=f"wqk{i}", name=f"wqk{i}") for i in range(NDC)]
            wv = [res.tile([128, DQK], BF16, tag=f"wv{i}", name=f"wv{i}") for i in range(NDC)]
            wp = [res.tile([128, D], BF16, tag=f"wp{i}", name=f"wp{i}") for i in range(NRC)]
            bqk = res.tile([128, 8], F32, tag="bqk", name="bqk_sb")
            msk = [res.tile([128, 1024], BF16, tag=f"msk{i}", name=f"msk{i}") for i in range(2)]
            ones = res.tile([128, HD], F32, tag="ones", name="ones_sb")
            # q^T,k^T resident: tiles 0..3 = q e-blocks, 4..7 = k e-blocks
            qkT = [res.tile([128, S], BF16, tag=f"qkT{i}", name=f"qkT{i}") for i in range(8)]
            # v in natural layout, augmented with a ones column per head:
            # tile st: [128 j, 8*65] with cols h*65..h*65+63 = v_h, h*65+64 = 1
            vA = [res.tile([128, HLOC * (HD + 1)], BF16, tag=f"v{i}", name=f"v{i}")
                  for i in range(NST)]
            # normalized a^T (heads packed in pairs: head h -> tile h//2,
            # partitions (h%2)*64..)
            aT = [res.tile([128, S], BF16, tag=f"aT{i}", name=f"aT{i}") for i in range(NRC)]

            # boot loads spread across the three DMA paths so the first QKV
            # chain isn't serialized on one queue: SP ring carries xt,
            # ACT ring carries wqk, SWDGE (vector) carries the rest
            for i in range(NDC):
                nc.sync.dma_start(
                    out=xt[i][:, 0:512], in_=xt_e[i * 128:(i + 1) * 128, 0:512])
                nc.scalar.dma_start(
                    out=wqk[i][:, 0:DQK],
                    in_=wqk_e[i * 128:(i + 1) * 128, 0:DQK])
            for i in range(NDC):
                nc.scalar.dma_start(
                    out=wqk[i][:, DQK:2 * DQK],
                    in_=wqk_e[i * 128:(i + 1) * 128, DQK:2 * DQK])
            for q in range(1, 4):
                for i in range(NDC):
                    eng = nc.sync if i % 2 == 0 else nc.scalar
                    eng.dma_start(
                        out=xt[i][:, q * 512:(q + 1) * 512],
                        in_=xt_e[i * 128:(i + 1) * 128, q * 512:(q + 1) * 512])
            for i in range(NDC):
                nc.gpsimd.dma_start(out=wv[i][:], in_=wv_e[i * 128:(i + 1) * 128, :])
            for i in range(NRC):
                nc.gpsimd.dma_start(out=wp[i][:], in_=wp_e[i * 128:(i + 1) * 128, :])
            nc.gpsimd.dma_start(out=bqk[:], in_=bqk_e[:])
            for g in range(2):
                nc.gpsimd.dma_start(out=msk[g][:], in_=msk_e[g])
            nc.vector.memset(ones[:], 1.0)
            for st in range(NST):
                va = vA[st]
                nc.vector.memset(
                    va.rearrange("p (h c) -> p h c", c=HD + 1)[:, :, HD:HD + 1], 1.0)

            for _it in range(niter):
                # ---- phases B and C interleaved: emit each head-pair's
                # attention right after its q/k/v inputs, so B's matmuls fill
                # the PE stalls left by C's ACT(exp)-bound stretches.
                # PSUM budget: pq 1 + pv 1 + psS 2x2 + psA 2x1 = 8 banks.
                if "b" not in phases and "c" not in phases:
                    continue
                ctxS = tc.tile_pool(name="psS", bufs=2, space="PSUM")
                ctxA = tc.tile_pool(name="psA", bufs=2, space="PSUM")
                ctxT = tc.tile_pool(name="att", bufs=10)
                ctxF = tc.tile_pool(name="attf", bufs=8)
                ctxD = tc.tile_pool(name="dscr", bufs=8, space="DRAM")
                psS = ctxS.__enter__(); psA = ctxA.__enter__()
                att = ctxT.__enter__(); attf = ctxF.__enter__()
                dscr = ctxD.__enter__()
                with tc.tile_pool(name="psB", bufs=1, space="PSUM") as psB, \
                     tc.tile_pool(name="psV", bufs=1, space="PSUM") as psV:

                    def emit_qk(eb):
                        if "b" not in phases:
                            return
                        for ss in range(NSS):
                            pq = psB.tile([128, 512], F32, tag="psB", name="pq")
                            for dc in range(NDC):
                                nc.tensor.matmul(
                                    pq[:],
                                    wqk[dc][:, eb * 128:(eb + 1) * 128],
                                    xt[dc][:, ss * 512:(ss + 1) * 512],
                                    start=(dc == 0), stop=(dc == NDC - 1))
                            nc.vector.tensor_scalar_add(
                                qkT[eb][:, ss * 512:(ss + 1) * 512],
                                pq[:], bqk[:, eb:eb + 1])

                    def emit_v(st):
                        if "b" not in phases:
                            return
                        pv = psV.tile([128, DQK], F32, tag="psV", name="pv")
                        for dc in range(NDC):
                            nc.tensor.matmul(
                                pv[:],
                                xt[dc][:, st * 128:(st + 1) * 128],
                                wv[dc][:],
                                start=(dc == 0), stop=(dc == NDC - 1))
                        nc.vector.tensor_copy(
                            vA[st].rearrange("p (h c) -> p h c", c=HD + 1)[:, :, 0:HD],
                            pv.rearrange("p (h c) -> p h c", c=HD))

                    def emit_att(m, ss, bcpool=None):
                        if "c" not in phases:
                            return
                        qt = qkT[m]
                        kt = qkT[4 + m]
                        njc = 4 * ss + 4
                        pas = [psA.tile([HD + 1, 512], F32, tag="psA",
                                        name=f"pa{e}") for e in range(2)]
                        for jg in range(njc // 2):
                            # diagonal groups: skip the fully-masked column
                            # prefix of each j-chunk.  diag0 (chunks 4ss,4ss+1):
                            # halves 512 and 384 wide (i>=0 / i>=128) in one
                            # fused [128, 896] tile; diag1 (4ss+2, 4ss+3):
                            # halves 256/256 at offset 256.
                            diag0 = (jg == 2 * ss)
                            diag1 = (jg == 2 * ss + 1)
                            if diag1:
                                Ws, cos = (256, 128), (256, 384)
                            elif diag0:
                                Ws, cos = (512, 384), (0, 128)
                            else:
                                Ws, cos = (512, 512), (0, 0)
                            tw = Ws[0] + Ws[1]
                            uo = (0, Ws[0])     # u-half offsets within tile
                            # diag1 tiles are small enough that both heads
                            # share one [128, 2*tw] psum tile -> one exp +
                            # one mask for the pair
                            eo = (0, 512) if diag1 else (0, 0)
                            if diag1:
                                ps = psS.tile([128, 1024], F32, tag="psS",
                                              name="psm")
                                pss = [ps, ps]
                            else:
                                pss = [psS.tile([128, tw], F32, tag="psS",
                                                name=f"ps{e}")
                                       for e in range(2)]
                            # interleaved so HW runs the two heads' QK on
                            # disjoint PE row groups concurrently
                            for u in range(2):
                                jc = 2 * jg + u
                                for e in range(2):
                                    po = e * 64
                                    o = eo[e] + uo[u]
                                    nc.tensor.matmul(
                                        pss[e][:, o:o + Ws[u]],
                                        kt[po:po + HD, jc * 128:(jc + 1) * 128],
                                        qt[po:po + HD,
                                           ss * 512 + cos[u]:
                                           ss * 512 + cos[u] + Ws[u]],
                                        start=True, stop=True)
                            pts = []
                            if diag1:
                                pt = att.tile([128, 2 * tw], BF16, tag="pt",
                                              name="ptm")
                                nc.scalar.activation(
                                    pt.rearrange("p (e i) -> p e i", i=tw),
                                    pss[0].rearrange("p (e i) -> p e i",
                                                     i=512)[:, :, 0:tw],
                                    mybir.ActivationFunctionType.Exp, scale=0.125)
                                nc.vector.tensor_mul(
                                    pt[:], pt[:], msk[1][:, 0:2 * tw])
                                pts = [pt, pt]
                            else:
                                for e in range(2):
                                    pt = att.tile([128, tw], BF16, tag="pt",
                                                  name=f"pt{e}")
                                    nc.scalar.activation(
                                        pt[:], pss[e][:],
                                        mybir.ActivationFunctionType.Exp,
                                        scale=0.125)
                                    if diag0:
                                        nc.vector.tensor_mul(
                                            pt[:], pt[:], msk[0][:, 0:tw])
                                    pts.append(pt)
                            ep = (0, tw) if diag1 else (0, 0)
                            for u in range(2):
                                jc = 2 * jg + u
                                for e in range(2):
                                    h = 2 * m + e
                                    o = ep[e] + uo[u]
                                    nc.tensor.matmul(
                                        pas[e][:, cos[u]:cos[u] + Ws[u]],
                                        vA[jc][:, h * (HD + 1):(h + 1) * (HD + 1)],
                                        pts[e][:, o:o + Ws[u]],
                                        start=(jc == 0), stop=(jc == njc - 1))
                        for e in range(2):
                            h = 2 * m + e
                            po = e * 64
                            pa = pas[e]
                            inv = attf.tile([128, 512], F32, tag="inv", name="inv")
                            nc.vector.reciprocal(inv[64:65, :], pa[HD:HD + 1, :])
                            if bcpool is not None:
                                # rank-1 PE broadcast: shorter latency chain
                                # than the DRAM bounce (matters for the tail)
                                bc = bcpool.tile([64, 512], F32, tag="psO",
                                                 name="bc")
                                nc.tensor.matmul(
                                    bc[:], ones[64:65, :], inv[64:65, :],
                                    start=True, stop=True)
                            else:
                                scr = dscr.tile([512], F32, tag="scr", name="scr")
                                nc.sync.dma_start(out=scr[:], in_=inv[64:65, :])
                                bc = attf.tile([64, 512], F32, tag="bc", name="bc")
                                nc.sync.dma_start(
                                    out=bc[:],
                                    in_=bass.AP(tensor=scr.tensor, offset=scr.offset,
                                                ap=[[0, 64]] + list(scr.ap)))
                            st2 = attf.tile([64, 512], BF16, tag="st2", name="st2")
                            nc.vector.tensor_mul(st2[:], pa[0:HD, :], bc[:])
                            nc.gpsimd.dma_start(
                                out=aT[m][po:po + HD, ss * 512:(ss + 1) * 512],
                                in_=st2[:])

                    emit_qk(0)
                    emit_qk(4)
                    for st in range(4):
                        emit_v(st)
                    emit_att(0, 0)
                    for st in range(4, 8):
                        emit_v(st)
                    emit_att(0, 1)
                    for st in range(8, 12):
                        emit_v(st)
                    emit_att(0, 2)
                    for st in range(12, 16):
                        emit_v(st)
                    emit_att(0, 3)
                    for m in range(1, 3):
                        emit_qk(m)
                        emit_qk(4 + m)
                        for ss in (3, 2, 1, 0):
                            emit_att(m, ss)
                    emit_qk(3)
                    emit_qk(7)

                # ---- last head pair interleaved with phase D: after each
                # (m=3, ss) the ss-column of aT is complete for all rc, so
                # D's ss-block runs while the next ss's attention is
                # ACT-bound.  PSUM: psS 4 + psA 2 + psO 2 = 8 banks.
                with tc.tile_pool(name="psO", bufs=2, space="PSUM") as psO, \
                     tc.tile_pool(name="osb", bufs=6) as osb:

                    def emit_d(ss):
                        if "d" not in phases:
                            return
                        for eb in range(8):
                            pout = psO.tile([128, 512], F32, tag="psO",
                                            name="po")
                            for rc in range(NRC):
                                nc.tensor.matmul(
                                    pout[:],
                                    wp[rc][:, eb * 128:(eb + 1) * 128],
                                    aT[rc][:, ss * 512:(ss + 1) * 512],
                                    start=(rc == 0), stop=(rc == NRC - 1))
                            ot = osb.tile([128, 512], F32, tag="ot", name="ot")
                            nc.vector.tensor_copy(ot[:], pout[:])
                            nc.scalar.dma_start(
                                out=out_e[eb * 128:(eb + 1) * 128,
                                          ss * 512:(ss + 1) * 512],
                                in_=ot[:])

                    for ss in (3, 2, 1, 0):
                        emit_att(3, ss)
                        emit_d(ss)
                ctxD.__exit__(None, None, None)
                ctxF.__exit__(None, None, None)
                ctxT.__exit__(None, None, None)
                ctxA.__exit__(None, None, None)
                ctxS.__exit__(None, None, None)

    nc.finalize()
    return nc


def get_graph(niter=1, phases="bcd"):
    key = (niter, phases)
    if key not in _CACHED:
        _CACHED[key] = _build(niter, phases)
    return _CACHED[key]


def _make_masks():
    """Masks for the wide exp tiles.

    A pt tile at (head, ss, jg) is [128, 1024]: half u (i columns
    u*512..u*512+511 of the tile) holds S^T for j-chunk jc=2*jg+u over the
    i-window [ss*512, ss*512+512).  Diagonal groups are jg with
    2*jg >= 4*ss, i.e. relative offsets (rr = jc - 4*ss) pairs (0,1), (2,3).
    Valid element: global j <= global i:
      jj + rr*128 <= ii  (ii in [0,512) relative to the i-super).
    Group g mask tile [128, 1024]: half u uses rr = 2*g + u.
    """
    out = np.zeros((2, 128, 1024), dtype=ml_dtypes.bfloat16)
    jj = np.arange(128)[:, None]
    ii = np.arange(512)[None, :]
    # msk[0]: fused diag0 tile [*, 896]: cols 0:512 chunk r=0 (valid j<=i),
    # cols 512:896 chunk r=1 over i in [128, 512) (valid jj+128 <= i)
    out[0, :, 0:512] = (jj <= ii).astype(ml_dtypes.bfloat16)
    ii384 = np.arange(128, 512)[None, :]
    out[0, :, 512:896] = (jj + 128 <= ii384).astype(ml_dtypes.bfloat16)
    # msk[1]: fused diag1 tile [*, 384]: cols 0:256 chunk r=2 over
    # i in [256, 512); cols 256:384 chunk r=3 over i in [384, 512)
    ii256 = np.arange(256, 512)[None, :]
    ii128 = np.arange(384, 512)[None, :]
    for e in range(2):      # same pattern for both heads of the pair
        out[1, :, e * 384:e * 384 + 256] = (jj + 256 <= ii256).astype(
            ml_dtypes.bfloat16)
        out[1, :, e * 384 + 256:e * 384 + 384] = (jj + 384 <= ii128).astype(
            ml_dtypes.bfloat16)
    return out


def _shard_inputs(x, c_attn_w, c_attn_b, c_proj_w, c_proj_b):
    bf = ml_dtypes.bfloat16
    msk = _make_masks()
    in_maps = []
    for c in range(8):
        b, hg = c // 2, c % 2
        qcols = slice(hg * DQK, hg * DQK + DQK)
        kcols = slice(D + hg * DQK, D + hg * DQK + DQK)
        vcols = slice(2 * D + hg * DQK, 2 * D + hg * DQK + DQK)
        wqk = np.concatenate(
            [c_attn_w[:, qcols], c_attn_w[:, kcols]], axis=1).astype(bf)
        wv = np.ascontiguousarray(c_attn_w[:, vcols]).astype(bf)
        wp = np.ascontiguousarray(
            c_proj_w[hg * DQK:hg * DQK + DQK, :]).astype(bf)
        bq = np.concatenate(
            [c_attn_b[qcols], c_attn_b[kcols]]).astype(np.float32)
        bqk = np.ascontiguousarray(bq.reshape(8, 128).T)  # [128, 8]
        xt = np.ascontiguousarray(x[b].T).astype(bf)
        in_maps.append({
            "xt": xt, "wqk": wqk, "wv": wv, "wp": wp,
            "bqk": bqk, "msk": msk,
        })
    return in_maps


def kernel(x, c_attn_w, c_attn_b, c_proj_w, c_proj_b, mask_self_attention):
    x = np.asarray(x)
    c_attn_w = np.asarray(c_attn_w)
    c_attn_b = np.asarray(c_attn_b)
    c_proj_w = np.asarray(c_proj_w)
    c_proj_b = np.asarray(c_proj_b)
    nc = get_graph()
    in_maps = _shard_inputs(x, c_attn_w, c_attn_b, c_proj_w, c_proj_b)
    res = run_bass_kernel_spmd(nc, in_maps, core_ids=list(range(8)))
    B = x.shape[0]
    # v-bias commutes through the projection: P_norm @ (v + 1*bv) adds the
    # constant bv @ Wp to every sequence position
    vconst = (c_attn_b[2 * D:3 * D].astype(np.float64)
              @ c_proj_w.astype(np.float64)).astype(np.float32)
    out = np.empty((B, S, D), dtype=np.float32)
    for b in range(B):
        acc = res.results[2 * b]["outT"] + res.results[2 * b + 1]["outT"]
        out[b] = acc.T + (c_proj_b.astype(np.float32) + vconst)[None, :]
    return out

